# revision 32
# baseline (speedup 1.0000x reference)
"""Bass/Trainium2 kernel for nn_HALTON_33277406609678 (ragged_sequence).

Reference computation:
    feat[b] = max over compacted-valid positions p in [s_b, e_b] of
              (p-th valid token of enc[b] if p < num_valid_b else 0)
    out = relu(feat @ W1 + b1) @ W2 + b2

pos_span values live in [0, 40), so at most the first ~40 valid tokens
of a row matter.  The host (cheap: indexing + dtype conversion only)
gathers those token rows per batch row into a dense fp16 tensor laid
out TRANSPOSED per D-chunk, so the device's span-max is a plain strided
reduce_max straight into the matmul layout -- no indirect DMA, no PE
transposes of gathered data.

Rank-tiered gather: rows are assigned to (core, slot-position) by
global span-length rank -- row of rank r goes to core r%8, position
r//8 -- so position i's slot budget T[i] = ceil4(len(rank 8i)) bounds
every core's row at that position.  All cores share one compiled
program (budgets are compile-time), but the gather shrinks ~2x vs
padding every row to the global max span.  Host unpermutes output rows.

Device pipeline: the first matmul is computed TRANSPOSED --
h^T[hc] = W1[:, hc-block]^T @ feat -- with the W1 block as the PE
stationary operand and feat (8 columns) streaming.  h^T [128, 8] lands
directly in PSUM, bias+relu is one per-partition Vector tensor_scalar
(b1 is partition-indexed), and the second matmul needs NO transposes
and NO PSUM->SBUF copies of h.

DMA (the measured bottleneck is HWDGE descriptor emission + per-DMA
completion latency, not wire bytes): all inputs ride ONE HWDGE queue
(sync) as 4 fat 128-descriptor transfers in strict priority order --
  G  = whole gather          (first: unlocks the span-max -> feat)
  WA = W2 + W1 h-blocks 0,1  (unlocks logits weights + first 2 groups)
  WB = W1 h-blocks 2,3,4
  WC = W1 h-block 5          (last bytes; only ONE group's matmuls +
                              relu + logits (~0.8us) trails its sem)
A single queue drains at full wire rate across all 16 SDMA engines and
FIFO order gives true priority.  The output DMA shares the sync ring
(its DGE runs long after the input DGEs); b1 rides the gpsimd SWDGE
queue.

Sharding: pure data parallel -- 8 batch rows per core, head weights
replicated (fp16).  b2 is added on the host (64x128 adds).

Slot semantics (host): slot j of row b holds compacted position q=s+j:
  real token       if q <= e and q <  nv
  zero row         if q <= e and q >= nv   (reference pools zeros there)
  dup of slot 0    if q >  e                (padding; never raises max)
If s >= nv the whole span is zero rows -> feat = 0 and the device MLP
yields relu(b1) @ W2 organically; no host patching needed.
"""

import numpy as np

B, L, D, H, K = 64, 512, 768, 768, 128
NCORES = 8
RPC = B // NCORES          # rows per core
CH = D // 128              # 128-wide chunks of D / H (= 6)

_CACHE = {}


def _build_nc(tiers):
    import concourse.bass as bass
    import concourse.bacc as bacc
    import concourse.mybir as mybir
    import concourse.tile as tile
    from concourse.tile_rust import add_dep_helper
    from contextlib import ExitStack

    f16 = mybir.dt.float16
    f32 = mybir.dt.float32

    nc = bacc.Bacc(
        "TRN2", target_bir_lowering=False, debug=False, num_devices=NCORES
    )
    TSUM = sum(tiers)          # gather cols per D-chunk (ragged by position)
    HB = CH * 128              # W1 cols per h-block (all 6 k-chunks x 128)
    WAC = CH * K + 2 * HB      # wa cols: w2 then w1 blocks 0,1
    g_d = nc.dram_tensor("g", [128, CH * TSUM], f16, kind="ExternalInput")
    wa_d = nc.dram_tensor("wa", [128, WAC], f16, kind="ExternalInput")
    wb_d = nc.dram_tensor("wb", [128, 3 * HB], f16, kind="ExternalInput")
    wc_d = nc.dram_tensor("wc", [128, 1 * HB], f16, kind="ExternalInput")
    b1_d = nc.dram_tensor("b1", [128, CH], f32, kind="ExternalInput")
    out_d = nc.dram_tensor("out", [RPC, K], f16, kind="ExternalOutput")

    with tile.TileContext(nc) as tc, ExitStack() as ctx:
        cpool = ctx.enter_context(tc.tile_pool(name="const", bufs=1))
        spool = ctx.enter_context(tc.tile_pool(name="scratch", bufs=2))
        ppool_h = ctx.enter_context(tc.tile_pool(name="ph", bufs=6, space="PSUM"))
        ppool_l = ctx.enter_context(tc.tile_pool(name="pl", bufs=1, space="PSUM"))

        # ---- DMA issue: one priority queue for all inputs ----------------
        g_sb = cpool.tile([128, CH * TSUM], f16, tag="g")
        g_i = nc.sync.dma_start(g_sb[:], g_d[:])
        wa_sb = cpool.tile([128, WAC], f16, tag="wa")
        wa_i = nc.sync.dma_start(wa_sb[:], wa_d[:])
        add_dep_helper(wa_i.ins, g_i.ins, sync=False,
                       reason="ring FIFO: gather first, then w2+w1b01")
        wb_sb = cpool.tile([128, 3 * HB], f16, tag="wb")
        wb_i = nc.sync.dma_start(wb_sb[:], wb_d[:])
        add_dep_helper(wb_i.ins, wa_i.ins, sync=False,
                       reason="ring FIFO: w1b234 after w1b01")
        wc_sb = cpool.tile([128, 1 * HB], f16, tag="wc")
        wc_i = nc.sync.dma_start(wc_sb[:], wc_d[:])
        add_dep_helper(wc_i.ins, wb_i.ins, sync=False,
                       reason="ring FIFO: w1b5 last")

        b1_sb = cpool.tile([128, CH], f32, tag="b1")
        nc.gpsimd.dma_start(b1_sb[:], b1_d[:])

        def w1_slice(hc, kc):
            if hc < 2:
                tile_, blk = wa_sb, hc
                base = CH * K
            elif hc < 5:
                tile_, blk, base = wb_sb, hc - 2, 0
            else:
                tile_, blk, base = wc_sb, 0, 0
            off = base + blk * HB + kc * 128
            return tile_[:, off:off + 128]

        # ---- span max per slot-position --------------------------------
        # g cols per half: (c, concat_i T[i] slots); feat col = c*RPC + i
        CHH = CH // 2
        feat_a = cpool.tile([128, CHH * RPC], f16, tag="fa")
        feat_b = cpool.tile([128, CHH * RPC], f16, tag="fb")
        # group equal-budget positions into single 4D reduces
        groups = []
        for i, t in enumerate(tiers):
            if groups and groups[-1][2] == t:
                groups[-1][1] += 1
            else:
                groups.append([i, 1, t])
        for half, ft in ((0, feat_a), (1, feat_b)):
            gh = g_sb[:, half * CHH * TSUM:(half + 1) * CHH * TSUM]
            gh3 = gh.rearrange("p (c t) -> p c t", c=CHH)
            ft3 = ft[:].rearrange("p (c r) -> p c r", c=CHH)
            off = 0
            for i0, n, t in groups:
                nc.vector.reduce_max(
                    ft3[:, :, i0:i0 + n],
                    gh3[:, :, off:off + n * t]
                    .rearrange("p c (x j) -> p c x j", x=n),
                    axis=mybir.AxisListType.X,
                )
                off += n * t

        def feat_slice(kc):
            t = feat_a if kc < CHH else feat_b
            c = kc if kc < CHH else kc - CHH
            return t[:, c * RPC:(c + 1) * RPC]

        # ---- per h-block: h^T = W1b^T @ feat -> relu -> logits accum -----
        l_ps = ppool_l.tile([RPC, K], f32, tag="l")
        for hc in range(CH):
            hps = ppool_h.tile([128, RPC], f32, tag="hps")
            for kc in range(CH):
                nc.tensor.matmul(
                    out=hps[:],
                    lhsT=w1_slice(hc, kc),
                    rhs=feat_slice(kc),
                    start=(kc == 0),
                    stop=(kc == CH - 1),
                )
            ht = spool.tile([128, RPC], f16, tag="ht")
            nc.vector.tensor_scalar(
                out=ht[:], in0=hps[:], scalar1=b1_sb[:, hc:hc + 1],
                scalar2=0.0,
                op0=mybir.AluOpType.add, op1=mybir.AluOpType.max,
            )
            nc.tensor.matmul(
                out=l_ps[:],
                lhsT=ht[:],
                rhs=wa_sb[:, hc * K:(hc + 1) * K],
                start=(hc == 0),
                stop=(hc == CH - 1),
            )

        out_sb = spool.tile([RPC, K], f16, tag="out")
        nc.vector.tensor_copy(out_sb[:], l_ps[:])
        oi = nc.sync.dma_start(out_d[:], out_sb[:])
        add_dep_helper(oi.ins, wc_i.ins, sync=False,
                       reason="ring FIFO: out after all input DMAs")

    nc.compile()
    return nc


def _get_nc(tiers):
    key = tuple(tiers)
    if key not in _CACHE:
        _CACHE[key] = _build_nc(key)
    return _CACHE[key]


def _plan_rows(pos_span):
    """Rank rows by span length; rank r -> core r%8, position r//8.

    Returns (perm [B] row index per (core, position), tiers [RPC]).
    """
    span = np.asarray(pos_span).astype(np.int64)
    lens = span[:, 1] - span[:, 0] + 1
    order = np.argsort(-lens, kind="stable")          # desc by span length
    perm = np.empty((NCORES, RPC), dtype=np.int64)
    for r, row in enumerate(order):
        perm[r % NCORES, r // NCORES] = row
    # quantize to 3 budget levels (positions 0-1 / 2-4 / 5-7) so the
    # device needs only 3 reduce instructions per gather half
    lvl = [0, 0, 1, 1, 1, 2, 2, 2]
    bounds = [int(lens[order[0]]), int(lens[order[2 * NCORES]]),
              int(lens[order[5 * NCORES]])]
    bounds = [max(4, (t + 3) // 4 * 4) for t in bounds]
    tiers = [bounds[lvl[i]] for i in range(RPC)]
    return perm, tiers


def _host_gather(enc16, valid_mask, pos_span, SLOTS):
    """Dense [B, SLOTS] token values per the slot semantics above -> fp16."""
    v = np.asarray(valid_mask).astype(np.int64) == 1          # [B, L]
    span = np.asarray(pos_span).astype(np.int64)              # [B, 2]
    s, e = span[:, 0], span[:, 1]
    nv = v.sum(axis=1)                                        # num valid per row
    order = np.argsort(~v, axis=1, kind="stable")             # valid tokens first
    q = s[:, None] + np.arange(SLOTS)[None, :]                # rank per slot
    qc = np.where(q <= e[:, None], q, s[:, None])             # padding -> slot 0
    use_zero = qc >= nv[:, None]                              # [B, SLOTS]
    toks = np.take_along_axis(order, np.minimum(qc, L - 1), axis=1)
    vals = enc16[np.arange(B)[:, None], toks]                 # [B, SLOTS, D]
    vals[use_zero] = np.float16(0.0)
    return vals


def _make_in_maps(inputs):
    enc16 = np.asarray(inputs["encoder_layers"], dtype=np.float32).astype(np.float16)
    W1 = np.asarray(inputs["W1"], dtype=np.float32)
    b1 = np.asarray(inputs["b1"], dtype=np.float32)
    W2 = np.asarray(inputs["W2"], dtype=np.float32)

    perm, tiers = _plan_rows(inputs["pos_span"])
    SLOTS = tiers[0]
    TSUM = sum(tiers)
    vals = _host_gather(enc16, inputs["valid_mask"], inputs["pos_span"], SLOTS)

    # device layouts: partition = d % 128
    w1_blocks = (W1.astype(np.float16).reshape(CH, 128, CH, 128)
                 .transpose(2, 1, 0, 3))                      # [hc, p, kc, col]
    w1_blocks = w1_blocks.reshape(CH, 128, CH * 128)
    w2_dev = (W2.astype(np.float16).reshape(CH, 128, K)
              .transpose(1, 0, 2).reshape(128, CH * K))
    wa = np.ascontiguousarray(np.concatenate(
        [w2_dev, w1_blocks[0], w1_blocks[1]], axis=1))
    wb = np.ascontiguousarray(np.concatenate(
        [w1_blocks[2], w1_blocks[3], w1_blocks[4]], axis=1))
    wc = np.ascontiguousarray(w1_blocks[5])
    b1_dev = np.ascontiguousarray(b1.reshape(CH, 128).T)      # [128, CH] f32

    in_maps = []
    for c in range(NCORES):
        rows = perm[c]                                        # [RPC] batch rows
        # per position i keep tiers[i] slots: [RPC rows ragged, D]
        segs = [vals[rows[i], :tiers[i]] for i in range(RPC)] # [t_i, D] each
        rag = np.concatenate(segs, axis=0)                    # [TSUM, D]
        # g[d%128, (c, t)] = rag[t, d]
        g = (rag.T                                            # [D, TSUM]
             .reshape(CH, 128, TSUM)
             .transpose(1, 0, 2)
             .reshape(128, CH * TSUM))
        in_maps.append({
            "g": np.ascontiguousarray(g),
            "b1": b1_dev, "wa": wa, "wb": wb, "wc": wc,
        })
    return in_maps, tiers, perm


def _apply_compiler_flags():
    import os
    maxsem = os.environ.get("BASS_MAX_SEM_NUM")
    if not maxsem:
        return
    from concourse.compiler_utils import get_compiler_flags, set_compiler_flags
    flags = get_compiler_flags()
    if "--max-sem-num" not in flags:
        set_compiler_flags(flags + ["--max-sem-num", maxsem])


def kernel(**inputs):
    from concourse.bass_utils import run_bass_kernel_spmd

    _apply_compiler_flags()
    in_maps, tiers, perm = _make_in_maps(inputs)
    nc = _get_nc(tiers)
    res = run_bass_kernel_spmd(nc, in_maps, list(range(NCORES)))
    out = np.empty((B, K), dtype=np.float32)
    for c in range(NCORES):
        out[perm[c]] = res.results[c]["out"].astype(np.float32)

    b2 = np.asarray(inputs["b2"], dtype=np.float32)
    return (out + b2[None, :]).astype(np.float32)
